# revision 12
# baseline (speedup 1.0000x reference)
"""GNN edge-softmax attention kernel for Trainium2 (8 NeuronCores) — V7.

Strategy (stream-packed, table-free, no collectives):
  Run 1 (pass A, src-range sharding): core c owns src nodes
  [c*6250,(c+1)*6250) and ALL edges with src in that range, so the
  per-src segment sum is complete locally (no AllReduce). Host packs
  fp8 feature columns nf[src_e], nf[dest_e] per edge slot (pure
  np.take data movement); the device projects q/k per 128-edge chunk
  with one fp8xbf16 matmul each, multiplies elementwise (DVE/Pool),
  head-reduces, adds mask, exps (Act), and accumulates seg[s,h] into
  a single PSUM bank via one-hot matmuls chained per src tile
  (uniform 18 chunks/tile across cores). Outputs per-edge ex (bf16)
  and per-node seg sums; host reassembles (concat/permute only).

  Run 2 (pass B, dest-range sharding): host packs bf16 nf[src_e]
  stream, relayed ex per slot, and seg[src_e] per slot (np.take).
  Device computes w = ex * 1/seg (reciprocal on device), projects
  v per chunk (d-major layout), wt = v*w, scatter-adds into accT
  [f',dest-row] PSUM per dest tile via flipped one-hot matmuls with
  narrow data-derived windows, then applies Wo directly from accT
  (no transpose) and streams out rows.
"""

import os

import numpy as np

import concourse.bacc as bacc
import concourse.bass as bass
import concourse.mybir as mybir
import concourse.tile as tile
from concourse.bass_utils import run_bass_kernel_spmd

N = 50000
F = 128
H = 8
D = 16
E = 800000
NCORES = 8
RPC = N // NCORES            # 6250 nodes per core
GB = 32                      # chunks per group
SHIFT = 8.0
MASK_PAD = -60.0

# run 1 (src-sharded) uniform chunk grid
NLT1 = 50                    # local src tiles (range may straddle 49/50)
NCPT1 = 18                   # chunks per tile (max tile edges <= 18*128)
CH1 = NLT1 * NCPT1           # 900
CH1P = ((CH1 + GB - 1) // GB) * GB   # 928 (dump-tile pads at tail)

# run 2 (dest-sharded) uniform chunk grid
NLT2 = 49                    # local dest tiles (6250 = 48*128+106)
NCPT2 = 18
CH2 = NLT2 * NCPT2           # 882
CH2P = ((CH2 + GB - 1) // GB) * GB   # 896

LAST_NC1 = None
LAST_NC2 = None

f32 = mybir.dt.float32
bf16 = mybir.dt.bfloat16
fp8 = mybir.dt.float8e4
ALU = mybir.AluOpType
ACTF = mybir.ActivationFunctionType

NP_BF16 = mybir.dt.np(bf16)
NP_FP8 = mybir.dt.np(fp8)

# DVE/Pool work split knobs
PROD_POOL = {1, 3, 4, 5, 7}  # which 4-chunk prod slices go to Pool (of 8)
WT_POOL = {1, 4, 7}          # run2 wt slices on Pool (of 8)


def _gen_run1(biases):
    has_bq, has_bk = biases
    nc = bacc.Bacc(None, target_bir_lowering=False)
    sstream = nc.dram_tensor("sstream", [128, CH1P * 128], fp8, kind="ExternalInput")
    dstream = nc.dram_tensor("dstream", [128, CH1P * 128], fp8, kind="ExternalInput")
    WqT = nc.dram_tensor("WqT", [128, 128], bf16, kind="ExternalInput")
    WkT = nc.dram_tensor("WkT", [128, 128], bf16, kind="ExternalInput")
    bqk = nc.dram_tensor("bqk", [1, 256], bf16, kind="ExternalInput")
    ohstream = nc.dram_tensor("ohstream", [128, CH1P * 128], fp8, kind="ExternalInput")
    maskp = nc.dram_tensor("maskp", [128, CH1P * H], f32, kind="ExternalInput")
    exout = nc.dram_tensor("exout", [128, CH1P * H], bf16, kind="ExternalOutput")
    segout = nc.dram_tensor("segout", [128, (NLT1 + 1) * H], f32, kind="ExternalOutput")

    NG = CH1P // GB
    with tile.TileContext(nc) as tc:
        with (
            tc.tile_pool(name="meta", bufs=1) as mp,
            tc.tile_pool(name="strm", bufs=3) as sp,
            tc.tile_pool(name="prod", bufs=2) as pp,
            tc.tile_pool(name="work", bufs=3) as wp,
            tc.tile_pool(name="oh", bufs=6) as op,
            tc.tile_pool(name="psq", bufs=3, space="PSUM") as psq,
            tc.tile_pool(name="psk", bufs=2, space="PSUM") as psk,
            tc.tile_pool(name="pseg", bufs=1, space="PSUM") as pseg,
        ):
            wq = mp.tile([128, 128], bf16)
            wk = mp.tile([128, 128], bf16)
            mask_sb = mp.tile([128, CH1P * H], f32)
            nc.sync.dma_start(out=wq[:], in_=WqT[:])
            nc.sync.dma_start(out=wk[:], in_=WkT[:])
            nc.sync.dma_start(out=mask_sb[:], in_=maskp[:])
            if has_bq or has_bk:
                ones1 = mp.tile([1, 128], bf16)
                bias_sb = mp.tile([1, 256], bf16)
                nc.vector.memset(ones1[:], 1.0)
                nc.sync.dma_start(out=bias_sb[:], in_=bqk[:])

            segps = pseg.tile([128, (NLT1 + 1) * H], f32)

            for g in range(NG):
                ssb = sp.tile([128, GB, 128], fp8, tag="ssb")
                dsb = sp.tile([128, GB, 128], fp8, tag="dsb")
                ohsb = sp.tile([128, GB, 128], fp8, tag="ohsb")
                nc.sync.dma_start(out=ssb[:], in_=sstream[:, g * GB * 128:(g + 1) * GB * 128])
                nc.sync.dma_start(out=dsb[:], in_=dstream[:, g * GB * 128:(g + 1) * GB * 128])
                nc.sync.dma_start(out=ohsb[:], in_=ohstream[:, g * GB * 128:(g + 1) * GB * 128])

                prodb = pp.tile([128, GB * 128], bf16, tag="prodb")
                for h8 in range(GB // 8):
                    pk8 = psk.tile([128, 8, 128], f32, tag="pk8")
                    for j in range(8):
                        cc = h8 * 8 + j
                        nc.tensor.matmul(out=pk8[:, j, :], lhsT=dsb[:, cc, :],
                                         rhs=wk[:], start=True,
                                         stop=not (has_bk))
                        if has_bk:
                            nc.tensor.matmul(out=pk8[:, j, :], lhsT=ones1[:],
                                             rhs=bias_sb[:, 128:256],
                                             start=False, stop=True)
                    kev = wp.tile([128, 8, 128], bf16, tag="kev")
                    nc.scalar.activation(out=kev[:], in_=pk8[:], func=ACTF.Copy)
                    for q4 in range(2):
                        pq = psq.tile([128, 4, 128], f32, tag="pq")
                        for j in range(4):
                            cc = h8 * 8 + q4 * 4 + j
                            nc.tensor.matmul(out=pq[:, j, :], lhsT=ssb[:, cc, :],
                                             rhs=wq[:], start=True,
                                             stop=not (has_bq))
                            if has_bq:
                                nc.tensor.matmul(out=pq[:, j, :], lhsT=ones1[:],
                                                 rhs=bias_sb[:, 0:128],
                                                 start=False, stop=True)
                        o0 = (h8 * 8 + q4 * 4) * 128
                        nc.vector.tensor_tensor(
                            out=prodb[:, o0:o0 + 512],
                            in0=pq[:].rearrange("p a b -> p (a b)"),
                            in1=kev[:, q4 * 4:(q4 + 1) * 4, :].rearrange(
                                "p a b -> p (a b)"),
                            op=ALU.mult)

                # head reduce: halving TT (Pool) then 8-wide reduce (DVE)
                ph = wp.tile([128, GB * 64], bf16, tag="ph")
                pr = prodb[:].rearrange("p (c h e d) -> p (c h) e d", h=H, e=2, d=8)
                nc.gpsimd.tensor_tensor(
                    out=ph[:].rearrange("p (c h d) -> p (c h) d", h=H, d=8),
                    in0=pr[:, :, 0, :], in1=pr[:, :, 1, :], op=ALU.add)
                scores = wp.tile([128, GB * H], f32, tag="scores")
                nc.vector.tensor_reduce(
                    out=scores[:],
                    in_=ph[:].rearrange("p (c h d) -> p (c h) d", h=H, d=8),
                    axis=mybir.AxisListType.X, op=ALU.add)

                sc2 = wp.tile([128, GB * H], f32, tag="sc2")
                nc.gpsimd.tensor_tensor(
                    out=sc2[:], in0=scores[:],
                    in1=mask_sb[:, g * GB * H:(g + 1) * GB * H], op=ALU.add)
                ex = wp.tile([128, GB * H], bf16, tag="ex")
                nc.scalar.activation(out=ex[:], in_=sc2[:], func=ACTF.Exp)
                nc.sync.dma_start(out=exout[:, g * GB * H:(g + 1) * GB * H], in_=ex[:])

                for j in range(GB):
                    c = g * GB + j
                    lt = min(c // NCPT1, NLT1)
                    if lt < NLT1:
                        first = (c % NCPT1 == 0)
                        last = (c % NCPT1 == NCPT1 - 1)
                    else:
                        first = (c == CH1)
                        last = (c == CH1P - 1)
                    nc.tensor.matmul(out=segps[:, lt * H:(lt + 1) * H],
                                     lhsT=ohsb[:, j, :], rhs=ex[:, j * H:(j + 1) * H],
                                     start=first, stop=last,
                                     skip_group_check=True)

            seg_sb = mp.tile([128, (NLT1 + 1) * H], f32)
            nc.scalar.activation(out=seg_sb[:], in_=segps[:], func=ACTF.Copy)
            nc.sync.dma_start(out=segout[:], in_=seg_sb[:])
    nc.compile()
    return nc


def _gen_run2(windows, has_bv, has_bo):
    nc = bacc.Bacc(None, target_bir_lowering=False)
    sstream = nc.dram_tensor("sstream", [128, CH2P * 128], bf16, kind="ExternalInput")
    WvT = nc.dram_tensor("WvT", [128, 128], bf16, kind="ExternalInput")
    WoT = nc.dram_tensor("WoT", [128, 128], bf16, kind="ExternalInput")
    bvo = nc.dram_tensor("bvo", [1, 256], bf16, kind="ExternalInput")
    ohstream = nc.dram_tensor("ohstream", [128, CH2P * 128], fp8, kind="ExternalInput")
    exB = nc.dram_tensor("exB", [128, CH2P * H], bf16, kind="ExternalInput")
    segsel = nc.dram_tensor("segsel", [128, CH2P * H], bf16, kind="ExternalInput")
    outd = nc.dram_tensor("outd", [NLT2 * 128, 128], f32, kind="ExternalOutput")

    NG = CH2P // GB
    with tile.TileContext(nc) as tc:
        with (
            tc.tile_pool(name="meta", bufs=1) as mp,
            tc.tile_pool(name="strm", bufs=3) as sp,
            tc.tile_pool(name="work", bufs=4) as wp,
            tc.tile_pool(name="oh", bufs=6) as op,
            tc.tile_pool(name="out", bufs=3) as outp,
            tc.tile_pool(name="psv", bufs=2, space="PSUM") as psv,
            tc.tile_pool(name="pacc", bufs=2, space="PSUM") as pacc,
            tc.tile_pool(name="pso", bufs=2, space="PSUM") as pso,
        ):
            wv = mp.tile([128, 128], bf16)
            wo = mp.tile([128, 128], bf16)
            exB_sb = mp.tile([128, CH2P * H], bf16)
            seg_sb = mp.tile([128, CH2P * H], bf16)
            nc.sync.dma_start(out=wv[:], in_=WvT[:])
            nc.sync.dma_start(out=wo[:], in_=WoT[:])
            nc.sync.dma_start(out=exB_sb[:], in_=exB[:])
            nc.sync.dma_start(out=seg_sb[:], in_=segsel[:])
            if has_bv or has_bo:
                ones1 = mp.tile([1, 128], bf16)
                bias_sb = mp.tile([1, 256], bf16)
                nc.vector.memset(ones1[:], 1.0)
                nc.sync.dma_start(out=bias_sb[:], in_=bvo[:])

            # w = exB / segsel, on device
            inv_sb = mp.tile([128, CH2P * H], bf16)
            with nc.allow_low_precision(reason="bf16 softmax weights, tol 2e-2"):
                nc.vector.reciprocal(out=inv_sb[:], in_=seg_sb[:])
            wgt_sb = mp.tile([128, CH2P * H], bf16)
            nc.vector.tensor_tensor(out=wgt_sb[:], in0=exB_sb[:],
                                    in1=inv_sb[:], op=ALU.mult)

            acc = None
            cur_lt = -1
            for g in range(NG):
                ssb = sp.tile([128, GB, 128], bf16, tag="ssb")
                ohsb = sp.tile([128, GB, 128], fp8, tag="ohsb")
                nc.sync.dma_start(out=ssb[:], in_=sstream[:, g * GB * 128:(g + 1) * GB * 128])
                nc.sync.dma_start(out=ohsb[:], in_=ohstream[:, g * GB * 128:(g + 1) * GB * 128])

                wtb = wp.tile([128, GB * 128], bf16, tag="wtb")
                for h8 in range(GB // 8):
                    pv8 = psv.tile([128, 8, 128], f32, tag="pv8")
                    for j in range(8):
                        cc = h8 * 8 + j
                        nc.tensor.matmul(out=pv8[:, j, :], lhsT=ssb[:, cc, :],
                                         rhs=wv[:], start=True,
                                         stop=not has_bv)
                        if has_bv:
                            nc.tensor.matmul(out=pv8[:, j, :], lhsT=ones1[:],
                                             rhs=bias_sb[:, 0:128],
                                             start=False, stop=True)
                    vev = wp.tile([128, 8, 128], bf16, tag="vev")
                    nc.scalar.activation(out=vev[:], in_=pv8[:], func=ACTF.Copy)
                    c0 = g * GB + h8 * 8
                    wslice = bass.AP(
                        wgt_sb.tensor, wgt_sb.offset + c0 * H,
                        [wgt_sb.ap[0]] + [[H, 8], [0, D], [1, H]])
                    o0 = h8 * 1024
                    nc.vector.tensor_tensor(
                        out=wtb[:, o0:o0 + 1024].rearrange(
                            "p (c d h) -> p c d h", d=D, h=H),
                        in0=vev[:].rearrange("p a (d h) -> p a d h", h=H),
                        in1=wslice, op=ALU.mult)

                for j in range(GB):
                    c = g * GB + j
                    lt = min(c // NCPT2, NLT2)
                    in_tile_i = c % NCPT2 if lt < NLT2 else (c - CH2)
                    if lt != cur_lt:
                        # close out previous tile
                        if acc is not None and cur_lt < NLT2:
                            _emit_proj(nc, mp, outp, pso, acc, wo, cur_lt, outd,
                                       has_bo, bias_sb if (has_bv or has_bo) else None,
                                       ones1 if (has_bv or has_bo) else None)
                        acc = pacc.tile([128, 128], f32, tag="acc")
                        cur_lt = lt
                    if lt < NLT2 and in_tile_i > 0:
                        a, W = windows[in_tile_i]
                    else:
                        a, W = 0, 128
                    first = in_tile_i == 0
                    nc.tensor.matmul(
                        out=acc[:, a:a + W],
                        lhsT=wtb[:, j * 128:(j + 1) * 128],
                        rhs=ohsb[:, j, a:a + W],
                        start=first, stop=(in_tile_i == NCPT2 - 1 or
                                           (lt == NLT2 and c == CH2P - 1)),
                        skip_group_check=True)
            if acc is not None and cur_lt < NLT2:
                _emit_proj(nc, mp, outp, pso, acc, wo, cur_lt, outd,
                           has_bo, bias_sb if (has_bv or has_bo) else None,
                           ones1 if (has_bv or has_bo) else None)
    nc.compile()
    return nc


def _emit_proj(nc, mp, outp, pso, acc, wo, lt, outd, has_bo, bias_sb, ones1):
    accsb = outp.tile([128, 128], bf16, tag="accsb")
    nc.scalar.activation(out=accsb[:], in_=acc[:], func=ACTF.Copy)
    po = pso.tile([128, 128], f32, tag="po")
    nc.tensor.matmul(out=po[:], lhsT=accsb[:], rhs=wo[:],
                     start=True, stop=not has_bo)
    if has_bo:
        nc.tensor.matmul(out=po[:], lhsT=ones1[:], rhs=bias_sb[:, 128:256],
                         start=False, stop=True)
    osb = outp.tile([128, 128], f32, tag="osb")
    nc.scalar.activation(out=osb[:], in_=po[:], func=ACTF.Copy)
    nc.sync.dma_start(out=outd[lt * 128:(lt + 1) * 128, :], in_=osb[:])


# ---------------------------------------------------------------------------
# host prep
# ---------------------------------------------------------------------------

def _prep_run1(src, dest, mask, nf8T):
    """Per-core pass-A inputs. Returns (inputs, gmaps, tile0s)."""
    ins, gmaps, tile0s = [], [], []
    for c in range(NCORES):
        base = c * RPC
        eids = np.where((src >= base) & (src < base + RPC))[0]
        order = np.argsort(src[eids], kind="stable")
        es = eids[order]
        ssrc = src[es]
        tile0 = base // 128
        lt = ssrc // 128 - tile0
        slot_src = np.zeros(CH1P * 128, np.int64)
        slot_dst = np.zeros(CH1P * 128, np.int64)
        slot_loc = np.zeros(CH1P * 128, np.int64)
        slot_ok = np.zeros(CH1P * 128, bool)
        slot_msk = np.full((CH1P * 128, H), MASK_PAD, np.float32)
        gmap = np.full(CH1P * 128, -1, np.int64)
        for t in range(NLT1):
            sel = es[lt == t]
            n = len(sel)
            assert n <= NCPT1 * 128, f"tile overflow {n}"
            b0 = t * NCPT1 * 128
            slot_src[b0:b0 + n] = src[sel]
            slot_dst[b0:b0 + n] = dest[sel]
            slot_loc[b0:b0 + n] = src[sel] % 128
            slot_ok[b0:b0 + n] = True
            slot_msk[b0:b0 + n] = mask[sel] - SHIFT
            gmap[b0:b0 + n] = sel
        oh = np.zeros((CH1P * 128, 128), NP_FP8)
        vi = np.where(slot_ok)[0]
        oh[vi, slot_loc[vi]] = 1.0
        ins.append({
            "sstream": np.ascontiguousarray(nf8T[:, slot_src]),
            "dstream": np.ascontiguousarray(nf8T[:, slot_dst]),
            "ohstream": np.ascontiguousarray(
                oh.reshape(CH1P, 128, 128).transpose(1, 0, 2)
                .reshape(128, CH1P * 128)),
            "maskp": np.ascontiguousarray(
                slot_msk.reshape(CH1P, 128, H).transpose(1, 0, 2)
                .reshape(128, CH1P * H)),
        })
        gmaps.append(gmap.reshape(CH1P, 128).T)     # [128, CH1P]
        tile0s.append(tile0)
    return ins, gmaps, tile0s


def _prep_run2(src, dest, ex_edge, seg_full, nf16T):
    ins, tile_windows = [], None
    # first pass: chunk assignment + window stats
    slot_data = []
    win_lo = np.full(NCPT2, 128, np.int64)
    win_hi = np.zeros(NCPT2, np.int64)
    for c in range(NCORES):
        base = c * RPC
        eids = np.where((dest >= base) & (dest < base + RPC))[0]
        order = np.argsort(dest[eids], kind="stable")
        es = eids[order]
        dl = dest[es] - base
        lt = dl // 128
        slot_e = np.full(CH2P * 128, -1, np.int64)
        for t in range(NLT2):
            sel = es[lt == t]
            n = len(sel)
            assert n <= NCPT2 * 128, f"dest tile overflow {n}"
            b0 = t * NCPT2 * 128
            slot_e[b0:b0 + n] = sel
            rows = dest[sel] - base - t * 128
            for i in range(1, NCPT2):
                seg = rows[i * 128:(i + 1) * 128]
                if len(seg):
                    win_lo[i] = min(win_lo[i], seg.min())
                    win_hi[i] = max(win_hi[i], seg.max() + 1)
        slot_data.append((slot_e, base))
    windows = [(0, 128)]
    for i in range(1, NCPT2):
        if win_hi[i] <= win_lo[i]:
            windows.append((0, 8))
            continue
        a = int(win_lo[i] // 8 * 8)
        W = int(min(128 - a, ((win_hi[i] - a + 7) // 8) * 8))
        windows.append((a, W))

    for c in range(NCORES):
        slot_e, base = slot_data[c]
        valid = slot_e >= 0
        sl = np.where(valid, slot_e, 0)
        ssrc = np.where(valid, src[sl], 0)
        oh = np.zeros((CH2P * 128, 128), NP_FP8)
        vi = np.where(valid)[0]
        oh[vi, (dest[slot_e[vi]] - base) % 128] = 1.0
        exs = np.zeros((CH2P * 128, H), np.float32)
        exs[valid] = ex_edge[slot_e[valid]]
        sgs = np.ones((CH2P * 128, H), np.float32)
        sgs[valid] = seg_full[src[slot_e[valid]]]
        ins.append({
            "sstream": np.ascontiguousarray(nf16T[:, ssrc]),
            "ohstream": np.ascontiguousarray(
                oh.reshape(CH2P, 128, 128).transpose(1, 0, 2)
                .reshape(128, CH2P * 128)),
            "exB": np.ascontiguousarray(
                exs.reshape(CH2P, 128, H).transpose(1, 0, 2)
                .reshape(128, CH2P * H).astype(NP_BF16)),
            "segsel": np.ascontiguousarray(
                sgs.reshape(CH2P, 128, H).transpose(1, 0, 2)
                .reshape(128, CH2P * H).astype(NP_BF16)),
        })
    return ins, windows


# ---------------------------------------------------------------------------
# numpy emulators (mirror device semantics incl. dtypes)
# ---------------------------------------------------------------------------

def _emulate_run1(ins):
    outs = []
    for m in ins:
        ss = m["sstream"].astype(np.float32)      # [128, CH*128]
        ds = m["dstream"].astype(np.float32)
        wq = m["WqT"].astype(np.float32)
        wk = m["WkT"].astype(np.float32)
        ohs = m["ohstream"].astype(np.float32)    # [128, CH*128]
        maskp = m["maskp"].astype(np.float32)
        q = np.einsum("ce,cf->ef", ss.reshape(128, CH1P * 128), wq)
        k = np.einsum("ce,cf->ef", ds.reshape(128, CH1P * 128), wk)
        k = k.astype(NP_BF16).astype(np.float32)
        prod = (q * k).astype(NP_BF16).astype(np.float32)
        pr = prod.reshape(CH1P * 128, H, 2, 8)
        ph = (pr[:, :, 0] + pr[:, :, 1]).astype(NP_BF16).astype(np.float32)
        scores = ph.sum(2)
        sc2 = (scores.reshape(CH1P, 128, H).transpose(1, 0, 2)
               .reshape(128, CH1P * H) + maskp)
        ex = np.exp(sc2).astype(NP_BF16)
        seg = np.zeros((128, (NLT1 + 1) * H), np.float32)
        exf = ex.astype(np.float32).reshape(128, CH1P, H)
        ohr = ohs.reshape(128, CH1P, 128)
        for c in range(CH1P):
            lt = min(c // NCPT1, NLT1)
            seg[:, lt * H:(lt + 1) * H] += ohr[:, c].T @ exf[:, c]
        outs.append({"exout": ex, "segout": seg})
    return outs


def _emulate_run2(ins, windows):
    outs = []
    for m in ins:
        ss = m["sstream"].astype(np.float32)
        wv = m["WvT"].astype(np.float32)
        wo = m["WoT"].astype(np.float32)
        ohs = m["ohstream"].astype(np.float32)
        exB = m["exB"].astype(np.float32)
        segsel = m["segsel"]
        inv = (1.0 / segsel.astype(np.float32)).astype(NP_BF16).astype(np.float32)
        wgt = (exB * inv).astype(NP_BF16).astype(np.float32)
        v = np.einsum("ce,cf->ef", ss, wv).astype(NP_BF16).astype(np.float32)
        wgtr = wgt.reshape(128, CH2P, H).transpose(1, 0, 2)   # [CH, 128, H]
        out = np.zeros((NLT2 * 128, 128), np.float32)
        for c in range(CH2P):
            lt = min(c // NCPT2, NLT2)
            if lt >= NLT2:
                continue
            vt = v[c * 128:(c + 1) * 128].reshape(128, D, H)
            wt = (vt * wgtr[c][:, None, :]).reshape(128, 128).astype(NP_BF16).astype(np.float32)
            oh = ohs.reshape(128, CH2P, 128)[:, c]
            accT = wt.T @ oh                       # [f', s]
            # accumulate into per-tile accT
            out[lt * 128:(lt + 1) * 128] += (accT.astype(np.float32).T)
        # apply Wo per tile with bf16 accT
        res = np.zeros_like(out)
        for t in range(NLT2):
            accT = out[t * 128:(t + 1) * 128].T.astype(NP_BF16).astype(np.float32)
            res[t * 128:(t + 1) * 128] = accT.T @ wo
        outs.append({"outd": res})
    return outs


# ---------------------------------------------------------------------------

def kernel(node_features, edge_index, attention_mask, Wq, bq, Wk, bk,
           Wv, bv, Wo, bo):
    global LAST_NC1, LAST_NC2
    node_features = np.asarray(node_features, np.float32)
    edge_index = np.asarray(edge_index)
    attention_mask = np.asarray(attention_mask, np.float32)
    Wq, bq = np.asarray(Wq, np.float32), np.asarray(bq, np.float32)
    Wk, bk = np.asarray(Wk, np.float32), np.asarray(bk, np.float32)
    Wv, bv = np.asarray(Wv, np.float32), np.asarray(bv, np.float32)
    Wo, bo = np.asarray(Wo, np.float32), np.asarray(bo, np.float32)
    src = edge_index[0].astype(np.int64)
    dest = edge_index[1].astype(np.int64)

    nf8T = np.ascontiguousarray(node_features.T.astype(NP_FP8))
    nf16T = np.ascontiguousarray(node_features.T.astype(NP_BF16))

    WqT25 = np.ascontiguousarray(Wq.T * 0.25).astype(NP_BF16)
    WkTc = np.ascontiguousarray(Wk.T).astype(NP_BF16)
    # d-major permutation for run2: f' = d*8+h <- f = h*16+d
    perm = (np.arange(128).reshape(H, D).T.reshape(-1))   # perm[d*8+h] = h*16+d
    WvTp = np.ascontiguousarray(Wv.T[:, perm]).astype(NP_BF16)
    WoTp = np.ascontiguousarray(Wo.T[perm, :]).astype(NP_BF16)
    bqk = np.concatenate([bq * 0.25, bk])[None, :].astype(NP_BF16)
    bvo = np.concatenate([bv[perm], bo])[None, :].astype(NP_BF16)
    has_bq, has_bk = bool(np.any(bq)), bool(np.any(bk))
    has_bv, has_bo = bool(np.any(bv)), bool(np.any(bo))

    in1, gmaps, tile0s = _prep_run1(src, dest, attention_mask, nf8T)
    for m in in1:
        m.update({"WqT": WqT25, "WkT": WkTc, "bqk": bqk})

    emulate = bool(os.environ.get("KERNEL_EMULATE"))
    if emulate:
        r1 = _emulate_run1(in1)
    else:
        nc1 = _gen_run1((has_bq, has_bk))
        LAST_NC1 = nc1
        r1 = run_bass_kernel_spmd(nc1, in1, core_ids=list(range(NCORES))).results

    # host: assemble seg_full + ex relay (pure data movement)
    seg_full = np.zeros((N, H), np.float32)
    ex_edge = np.zeros((E, H), np.float32)
    for c in range(NCORES):
        seg = np.asarray(r1[c]["segout"], np.float32)     # [128, 51*8]
        nodes = np.arange(c * RPC, (c + 1) * RPC)
        lt = nodes // 128 - tile0s[c]
        seg_full[nodes] = seg.reshape(128, NLT1 + 1, H)[nodes % 128, lt]
        exo = np.asarray(r1[c]["exout"].astype(np.float32)).reshape(128, CH1P, H)
        gm = gmaps[c]
        vmask = gm >= 0
        ex_edge[gm[vmask]] = exo[vmask]

    in2, windows = _prep_run2(src, dest, ex_edge, seg_full, nf16T)
    for m in in2:
        m.update({"WvT": WvTp, "WoT": WoTp, "bvo": bvo})

    if emulate:
        r2 = _emulate_run2(in2, windows)
    else:
        nc2 = _gen_run2(windows, has_bv, has_bo)
        LAST_NC2 = nc2
        r2 = run_bass_kernel_spmd(nc2, in2, core_ids=list(range(NCORES))).results

    out = np.concatenate(
        [np.asarray(r2[c]["outd"], np.float32)[:RPC] for c in range(NCORES)], 0)
    return out.astype(np.float32)
